# revision 20
# baseline (speedup 1.0000x reference)
"""Trainium2 Bass kernel for nn_CRF_1597727834175 (batched Viterbi decode).

reference, per batch b:
    v0 = [-10000]*128 with v0[0]=v0[3]=0
    for t in 0..T-1:
        scores[i,j] = v[i] + trans[i,j]          (f32)
        road[t][j]  = argmax_i scores[i,j]       (first index on ties)
        v[j]        = max_i scores[i,j] + emit[t,j]
    best_score = max_j v[j]; best_last = argmax_j v[j]
    backtrack: ptr=best_last; path[t] = road[t][ptr], t = T-1..0
returns (inputs, best_score, paths).

Device mapping (per core, 32 of 256 batches; j = current tag on partitions,
i = previous tag on the free dim):

  PE (PSUM accumulation, 3 matmuls per batch b -> sc_b[j,i]):
      ps  = sel_b.T @ vmaxT          (= vmax[b,i] broadcast to 128 parts)
      ps += sel_b.T @ emit[t-1]      (= v[b,i], exact f32 order max+emit)
      ps += Id.T    @ transT         (= v[b,i] + trans[i,j] == reference order)
  ACT: copies PSUM score groups ([128,512] = 4 batches) into scbig SBUF.
  DVE: one segmented reduce max -> vmax[128,32]; per-b max_index
      (first-occurrence semantics == jnp.argmax ties); u16->u8 trellis copy.
  PE transpose + ACT copy: vmax [128,32] -> vmaxT [32,128] for the next step.

Backpointer trellis lives in SBUF per chunk, DMA'd out as uint8.
Final max/argmax + backtrack run on the host (~0.03% of the FLOPs).
"""

import os
import numpy as np

NUM_LABELS = 128
BATCH, SEQ = 256, 1024
N_CORES = 8
B_LOC = BATCH // N_CORES  # 32
NEG = -10000.0

_CACHE = {}
LAST_EXEC_NS = None
LAST_RES = [None]


def _build(T, CH, dynamic=True):
    import concourse.bass as bass
    import concourse.bacc as bacc
    import concourse.mybir as mybir
    from concourse.tile import TileContext

    f32 = mybir.dt.float32
    u8 = mybir.dt.uint8
    u16 = mybir.dt.uint16
    MAX = mybir.AluOpType.max
    L = NUM_LABELS

    nc = bacc.Bacc("TRN2", target_bir_lowering=False, debug=False)

    emit_d = nc.dram_tensor("emit", [B_LOC, T, L], f32, kind="ExternalInput")
    transT_d = nc.dram_tensor("transT", [L, L], f32, kind="ExternalInput")
    sel_d = nc.dram_tensor("sel", [B_LOC, B_LOC * L], f32, kind="ExternalInput")
    ident_d = nc.dram_tensor("ident", [L, L], f32, kind="ExternalInput")
    trell_d = nc.dram_tensor("trell", [L, T, B_LOC], u8, kind="ExternalOutput")
    vfin_d = nc.dram_tensor("vfin", [B_LOC, L], f32, kind="ExternalOutput")

    n_chunks = T // CH
    assert n_chunks * CH == T and n_chunks >= 1

    with TileContext(nc) as tc:
        with (
            tc.tile_pool(name="const", bufs=1) as cpool,
            tc.tile_pool(name="emitp", bufs=2) as epool,
            tc.tile_pool(name="trellp", bufs=2) as tpool,
            tc.tile_pool(name="scp", bufs=2) as scpool,
            tc.tile_pool(name="vmx", bufs=2) as vpool,
            tc.tile_pool(name="vtp", bufs=1) as vtpool,
            tc.tile_pool(name="idxp", bufs=2) as ipool,
            tc.tile_pool(name="psg", bufs=6, space="PSUM") as psgpool,
            tc.tile_pool(name="pst", bufs=1, space="PSUM") as pstpool,
        ):
            transT = cpool.tile([L, L], f32)
            nc.sync.dma_start(out=transT[:], in_=transT_d[:])
            sel = cpool.tile([B_LOC, B_LOC * L], f32)
            nc.sync.dma_start(out=sel[:], in_=sel_d[:])
            ident = cpool.tile([L, L], f32)
            nc.sync.dma_start(out=ident[:], in_=ident_d[:])

            # PE Matmult supports a single sync-wait in codegen.  Absorb each
            # const-DMA wait into PE's vector clock with sacrificial matmuls
            # so the real matmuls only ever carry one wait.
            psd = pstpool.tile([1, 1], f32, tag="psd")
            nc.tensor.matmul(out=psd[:], lhsT=sel[:, 0:1], rhs=sel[:, 0:1],
                             start=True, stop=True)
            nc.tensor.matmul(out=psd[:], lhsT=ident[:, 0:1], rhs=ident[:, 0:1],
                             start=True, stop=True)
            nc.tensor.matmul(out=psd[:], lhsT=transT[:, 0:1], rhs=transT[:, 0:1],
                             start=True, stop=True)

            # vmaxT holds v (before emission add) transposed: fixed slot so the
            # PE->ACT->PE chain and the post-loop epilogue can reference it.
            vmaxT = vtpool.tile([B_LOC, L], f32, tag="vmaxT")
            nc.vector.memset(vmaxT[:], NEG)
            nc.vector.memset(vmaxT[:, 0:1], 0.0)
            nc.vector.memset(vmaxT[:, 3:4], 0.0)

            def do_step(tl, emit_sb, trell_sb, first):
                """One Viterbi step. emit_sb slice tl holds emit[t-1]."""
                scbig = scpool.tile([L, B_LOC, L], f32, tag="scbig")
                vmax = vpool.tile([L, B_LOC], f32, tag="vmax")
                idx8 = ipool.tile([L, B_LOC, 8], u16, tag="idx8")
                for g in range(8):
                    ps = psgpool.tile([L, 4 * L], f32, tag="psg")
                    for k in range(4):
                        b = 4 * g + k
                        sl = ps[:, k * L:(k + 1) * L]
                        nc.tensor.matmul(
                            out=sl, lhsT=sel[:, b * L:(b + 1) * L], rhs=vmaxT[:],
                            start=True, stop=False, skip_group_check=True)
                        if not first:
                            nc.tensor.matmul(
                                out=sl, lhsT=sel[:, b * L:(b + 1) * L],
                                rhs=emit_sb[:, tl, :],
                                start=False, stop=False, skip_group_check=True)
                        nc.tensor.matmul(
                            out=sl, lhsT=ident[:], rhs=transT[:],
                            start=False, stop=True, skip_group_check=True)
                    nc.scalar.copy(out=scbig[:, 4 * g:4 * (g + 1), :], in_=ps[:])
                nc.vector.tensor_reduce(
                    out=vmax[:], in_=scbig[:], axis=mybir.AxisListType.X, op=MAX)
                pst = pstpool.tile([B_LOC, L], f32, tag="pst")
                nc.tensor.transpose(out=pst[:], in_=vmax[:], identity=ident[:])
                nc.scalar.copy(out=vmaxT[:], in_=pst[:])
                for b in range(B_LOC):
                    nc.vector.max_index(
                        out=idx8[:, b, :],
                        in_max=vmax[:, b:b + 1].broadcast_to([L, 8]),
                        in_values=scbig[:, b, :])
                nc.vector.tensor_copy(trell_sb[:, tl, :], idx8[:, :, 0])

            def do_chunk(iv, first_chunk):
                """iv = chunk start step (int for static, ScalarValue for For_i).
                Emission window is [iv-1, iv+CH-1) so slice tl == emit[t-1]."""
                emit_sb = epool.tile([B_LOC, CH, L], f32, tag="emit")
                if first_chunk:
                    if CH > 1:
                        nc.sync.dma_start(out=emit_sb[:, 1:, :],
                                          in_=emit_d[:, 0:CH - 1, :])
                else:
                    nc.sync.dma_start(out=emit_sb[:],
                                      in_=emit_d[:, bass.ds(iv - 1, CH), :])
                trell_sb = tpool.tile([L, CH, B_LOC], u8, tag="trell")
                for tl in range(CH):
                    do_step(tl, emit_sb, trell_sb, first_chunk and tl == 0)
                if first_chunk:
                    nc.sync.dma_start(out=trell_d[:, 0:CH, :], in_=trell_sb[:])
                else:
                    nc.sync.dma_start(out=trell_d[:, bass.ds(iv, CH), :],
                                      in_=trell_sb[:])

            do_chunk(0, True)
            if n_chunks > 1:
                if dynamic:
                    with tc.For_i(CH, T, CH, staggered_reset=True) as iv:
                        do_chunk(iv, False)
                else:
                    for c in range(1, n_chunks):
                        do_chunk(c * CH, False)

            # epilogue: v_final = vmax[T-1] + emit[T-1]
            emit_last = epool.tile([B_LOC, 1, L], f32, tag="emitlast")
            nc.sync.dma_start(out=emit_last[:], in_=emit_d[:, T - 1:T, :])
            vfin_sb = vtpool.tile([B_LOC, L], f32, tag="vfin")
            nc.vector.tensor_add(vfin_sb[:], vmaxT[:], emit_last[:, 0, :])
            nc.sync.dma_start(out=vfin_d[:], in_=vfin_sb[:])

    nc.compile()
    return nc


def _strip_pe_own_waits(nc):
    """Remove Matmult waits on PE's own semaphore that are provably satisfied
    by program order.  Valid only for a fully static (loop-free) module: walk
    blocks in order, count each semaphore's increments, and drop a same-engine
    wait whose target value has already been reached earlier in the program.
    Walrus codegen only supports a single sync-wait per Matmult; Tile emits a
    redundant PE-own wait on PSUM slot reuse (in-order engine => always met).
    """
    import concourse.mybir as mybir

    counts = {}
    n_stripped = 0
    n_multi = 0
    for blk in nc.m.functions[0].blocks:
        for inst in blk.instructions:
            si = inst.sync_info
            if si is not None and si.on_wait:
                if type(inst).__name__ == "InstMatmult":
                    eng = str(inst.engine).split(".")[-1]
                    keep = []
                    for w in si.on_wait:
                        own = w.ant_name.split("_")[0] == eng
                        if (own and str(w.wait_mode).endswith("sem-ge-imm")
                                and counts.get(w.ant_name, 0) >= w.wait_value):
                            n_stripped += 1
                        else:
                            keep.append(w)
                    if len(keep) != len(si.on_wait):
                        inst.sync_info = mybir.SyncInfo(
                            on_wait=keep, on_update=list(si.on_update))
                    if len(keep) > 1:
                        n_multi += 1
            si = inst.sync_info
            if si is not None:
                for u in si.on_update:
                    if str(u.update_mode).endswith("sem-inc"):
                        counts[u.ant_name] = counts.get(u.ant_name, 0) + u.update_value
    return n_stripped, n_multi


def _get_nc(T=SEQ, CH=64, dynamic=False):
    key = (T, CH, dynamic)
    if key not in _CACHE:
        _CACHE[key] = _build(T, CH, dynamic)
    return _CACHE[key]


def _sel_matrix():
    sel = np.zeros((B_LOC, B_LOC * NUM_LABELS), dtype=np.float32)
    for b in range(B_LOC):
        sel[b, b * NUM_LABELS:(b + 1) * NUM_LABELS] = 1.0
    return sel


class _Runner:
    """Builds the Bass module and a cached sharded jit over the 8 cores.

    Mirrors concourse.bass2jax.run_bass_via_pjrt's multi-core path, but keeps
    the jitted callable so repeat invocations skip retrace/recompile.
    """

    def __init__(self, T):
        import jax
        import numpy as _np
        from jax.sharding import Mesh, PartitionSpec
        from jax.experimental.shard_map import shard_map
        import concourse.mybir as mybir
        from concourse import bass2jax

        bass2jax.install_neuronx_cc_hook()
        nc = _get_nc(T=T)
        self.nc = nc
        partition_name = (nc.partition_id_tensor.name
                          if nc.partition_id_tensor else None)
        in_names, out_names, out_avals = [], [], []
        for alloc in nc.m.functions[0].allocations:
            if not isinstance(alloc, mybir.MemoryLocationSet):
                continue
            name = alloc.memorylocations[0].name
            if alloc.kind == "ExternalInput":
                if name != partition_name:
                    in_names.append(name)
            elif alloc.kind == "ExternalOutput":
                out_names.append(name)
                out_avals.append(jax.core.ShapedArray(
                    tuple(alloc.tensor_shape), mybir.dt.np(alloc.dtype)))
        self.in_names, self.out_names, self.out_avals = in_names, out_names, out_avals
        n_params = len(in_names)
        bind_names = list(in_names + out_names)
        if partition_name is not None:
            bind_names.append(partition_name)
        bind_names = tuple(bind_names)

        def _body(*args):
            operands = list(args)
            if partition_name is not None:
                operands.append(bass2jax.partition_id_tensor())
            outs = bass2jax._bass_exec_p.bind(
                *operands,
                out_avals=tuple(out_avals),
                in_names=bind_names,
                out_names=tuple(out_names),
                lowering_input_output_aliases=(),
                sim_require_finite=True,
                sim_require_nnan=True,
                nc=nc,
            )
            return tuple(outs)

        devices = jax.devices()[:N_CORES]
        self.mesh = Mesh(_np.asarray(devices), ("core",))
        n_outs = len(out_names)
        in_specs = (PartitionSpec("core"),) * (n_params + n_outs)
        out_specs = (PartitionSpec("core"),) * n_outs
        self.jit = jax.jit(
            shard_map(_body, mesh=self.mesh, in_specs=in_specs,
                      out_specs=out_specs, check_rep=False),
            donate_argnums=tuple(range(n_params, n_params + n_outs)),
            keep_unused=True,
        )

    def run(self, per_core_inputs):
        """per_core_inputs: list (len 8) of dicts name->np array."""
        import numpy as _np
        concat_in = [
            _np.concatenate([per_core_inputs[c][n] for c in range(N_CORES)], axis=0)
            for n in self.in_names
        ]
        zeros = [
            _np.zeros((N_CORES * a.shape[0], *a.shape[1:]), a.dtype)
            for a in self.out_avals
        ]
        out_arrs = self.jit(*concat_in, *zeros)
        return {
            n: _np.asarray(out_arrs[i]).reshape(
                N_CORES, *self.out_avals[i].shape)
            for i, n in enumerate(self.out_names)
        }


_RUNNERS = {}


def _get_runner(T):
    if T not in _RUNNERS:
        _RUNNERS[T] = _Runner(T)
    return _RUNNERS[T]


def kernel(inputs: np.ndarray, trans: np.ndarray) -> tuple:
    inputs = np.ascontiguousarray(inputs, dtype=np.float32)
    trans = np.ascontiguousarray(trans, dtype=np.float32)
    B, T, L = inputs.shape
    transT = np.ascontiguousarray(trans.T)
    ident = np.eye(L, dtype=np.float32)
    sel = _sel_matrix()

    runner = _get_runner(T)
    per_core = [
        {"emit": inputs[c * B_LOC:(c + 1) * B_LOC], "transT": transT,
         "sel": sel, "ident": ident}
        for c in range(N_CORES)
    ]
    outs = runner.run(per_core)

    # host epilogue: final max/argmax + backtrack (exact first-index ties)
    vfin = outs["vfin"].reshape(B, L)
    scores = vfin.max(axis=1)
    ptr = vfin.argmax(axis=1).astype(np.int64)
    # trell: [8, 128, T, 32] -> [128, T, 256] (core-major batch order)
    trell = np.moveaxis(outs["trell"], 0, 2).reshape(NUM_LABELS, T, B)
    paths = np.empty((B, T), dtype=np.int32)
    ar = np.arange(B)
    for t in range(T - 1, -1, -1):
        ptr = trell[ptr, t, ar].astype(np.int64)
        paths[:, t] = ptr
    return inputs, scores, paths


# revision 26
# speedup vs baseline: 1.6428x; 1.6428x over previous
"""Trainium2 Bass kernel for nn_CRF_1597727834175 (batched Viterbi decode).

reference, per batch b:
    v0 = [-10000]*128 with v0[0]=v0[3]=0
    for t in 0..T-1:
        scores[i,j] = v[i] + trans[i,j]          (f32)
        road[t][j]  = argmax_i scores[i,j]       (first index on ties)
        v[j]        = max_i scores[i,j] + emit[t,j]
    best_score = max_j v[j]; best_last = argmax_j v[j]
    backtrack: ptr=best_last; path[t] = road[t][ptr], t = T-1..0
returns (inputs, best_score, paths).

Device mapping (per core, 32 of 256 batches; j = current tag on partitions,
i = previous tag on the free dim):

  PE (PSUM accumulation, 3 matmuls per batch b -> sc_b[j,i]):
      ps  = sel_b.T @ vmaxT          (= vmax[b,i] broadcast to 128 parts)
      ps += sel_b.T @ emit[t-1]      (= v[b,i], exact f32 order max+emit)
      ps += Id.T    @ transT         (= v[b,i] + trans[i,j] == reference order)
  ACT: copies PSUM score groups ([128,512] = 4 batches) into scbig SBUF.
  DVE: one segmented reduce max -> vmax[128,32]; per-b max_index
      (first-occurrence semantics == jnp.argmax ties); u16->u8 trellis copy.
  PE transpose + ACT copy: vmax [128,32] -> vmaxT [32,128] for the next step.

Backpointer trellis lives in SBUF per chunk, DMA'd out as uint8.
Final max/argmax + backtrack run on the host (~0.03% of the FLOPs).
"""

import os
import numpy as np

NUM_LABELS = 128
BATCH, SEQ = 256, 1024
N_CORES = 8
B_LOC = BATCH // N_CORES  # 32
NEG = -10000.0

_CACHE = {}
LAST_EXEC_NS = None
LAST_RES = [None]


def _build(T, CH, dynamic=True):
    import concourse.bass as bass
    import concourse.bacc as bacc
    import concourse.mybir as mybir
    from concourse.tile import TileContext

    f32 = mybir.dt.float32
    u8 = mybir.dt.uint8
    u16 = mybir.dt.uint16
    MAX = mybir.AluOpType.max
    L = NUM_LABELS

    nc = bacc.Bacc("TRN2", target_bir_lowering=False, debug=False)

    emit_d = nc.dram_tensor("emit", [B_LOC, T, L], f32, kind="ExternalInput")
    transT_d = nc.dram_tensor("transT", [L, L], f32, kind="ExternalInput")
    sel_d = nc.dram_tensor("sel", [B_LOC, B_LOC * L], f32, kind="ExternalInput")
    ident_d = nc.dram_tensor("ident", [L, L], f32, kind="ExternalInput")
    transT4_d = nc.dram_tensor("transT4", [L, 4 * L], f32, kind="ExternalInput")
    trell_d = nc.dram_tensor("trell", [L, T, B_LOC], u8, kind="ExternalOutput")
    vfin_d = nc.dram_tensor("vfin", [B_LOC, L], f32, kind="ExternalOutput")

    n_chunks = T // CH
    assert n_chunks * CH == T and n_chunks >= 1

    with TileContext(nc) as tc:
        with (
            tc.tile_pool(name="const", bufs=1) as cpool,
            tc.tile_pool(name="emitp", bufs=2) as epool,
            tc.tile_pool(name="trellp", bufs=2) as tpool,
            tc.tile_pool(name="scp", bufs=2) as scpool,
            tc.tile_pool(name="vmx", bufs=2) as vpool,
            tc.tile_pool(name="vtp", bufs=1) as vtpool,
            tc.tile_pool(name="idxp", bufs=2) as ipool,
            tc.tile_pool(name="psg", bufs=6, space="PSUM") as psgpool,
            tc.tile_pool(name="pst", bufs=1, space="PSUM") as pstpool,
        ):
            transT = cpool.tile([L, L], f32)
            nc.sync.dma_start(out=transT[:], in_=transT_d[:])
            sel = cpool.tile([B_LOC, B_LOC * L], f32)
            nc.sync.dma_start(out=sel[:], in_=sel_d[:])
            ident = cpool.tile([L, L], f32)
            nc.sync.dma_start(out=ident[:], in_=ident_d[:])
            transT4 = cpool.tile([L, 4 * L], f32)
            nc.sync.dma_start(out=transT4[:], in_=transT4_d[:])

            # PE Matmult supports a single sync-wait in codegen.  Absorb each
            # const-DMA wait into PE's vector clock with sacrificial matmuls
            # so the real matmuls only ever carry one wait.
            psd = pstpool.tile([1, 1], f32, tag="psd")
            nc.tensor.matmul(out=psd[:], lhsT=sel[:, 0:1], rhs=sel[:, 0:1],
                             start=True, stop=True)
            nc.tensor.matmul(out=psd[:], lhsT=ident[:, 0:1], rhs=ident[:, 0:1],
                             start=True, stop=True)
            nc.tensor.matmul(out=psd[:], lhsT=transT[:, 0:1], rhs=transT[:, 0:1],
                             start=True, stop=True)
            nc.tensor.matmul(out=psd[:], lhsT=transT4[:, 0:1], rhs=transT4[:, 0:1],
                             start=True, stop=True)

            # vmaxT holds v (before emission add) transposed: fixed slot so the
            # PE->ACT->PE chain and the post-loop epilogue can reference it.
            vmaxT = vtpool.tile([B_LOC, L], f32, tag="vmaxT")
            nc.vector.memset(vmaxT[:], NEG)
            nc.vector.memset(vmaxT[:, 0:1], 0.0)
            nc.vector.memset(vmaxT[:, 3:4], 0.0)

            def do_step(tl, emit_sb, trell_sb):
                """One Viterbi step t: sc_b = vT[b] + transT (PE PSUM accum),
                reduce+argmax on DVE, then vT <- transpose(vmax) + emit[t]."""
                scbig = scpool.tile([L, B_LOC, L], f32, tag="scbig")
                vmax = vpool.tile([L, B_LOC], f32, tag="vmax")
                idx8 = ipool.tile([L, B_LOC, 8], u16, tag="idx8")
                for g in range(8):
                    ps = psgpool.tile([L, 4 * L], f32, tag="psg")
                    # ps[j, (k,i)] = transT[j, i] (one wide matmul), then each
                    # slice accumulates v_b[i]; fl(trans+v) == fl(v+trans).
                    nc.tensor.matmul(
                        out=ps[:], lhsT=ident[:], rhs=transT4[:],
                        start=True, stop=False, skip_group_check=True)
                    for k in range(4):
                        b = 4 * g + k
                        nc.tensor.matmul(
                            out=ps[:, k * L:(k + 1) * L],
                            lhsT=sel[:, b * L:(b + 1) * L], rhs=vmaxT[:],
                            start=False, stop=True, skip_group_check=True)
                    nc.scalar.copy(out=scbig[:, 4 * g:4 * (g + 1), :], in_=ps[:])
                nc.vector.tensor_reduce(
                    out=vmax[:], in_=scbig[:], axis=mybir.AxisListType.X, op=MAX)
                pst = pstpool.tile([B_LOC, L], f32, tag="pst")
                nc.tensor.transpose(out=pst[:], in_=vmax[:], identity=ident[:])
                # vT <- vmax^T + emit[t]   (exact (max+emit) rounding order)
                nc.vector.tensor_add(vmaxT[:], pst[:], emit_sb[:, tl, :])
                for b in range(B_LOC):
                    nc.vector.max_index(
                        out=idx8[:, b, :],
                        in_max=vmax[:, b:b + 1].broadcast_to([L, 8]),
                        in_values=scbig[:, b, :])
                nc.vector.tensor_copy(trell_sb[:, tl, :], idx8[:, :, 0])

            def do_chunk(iv, first_chunk):
                """iv = chunk start step (int for static, ScalarValue for
                For_i). Emission slice tl == emit[iv + tl]."""
                emit_sb = epool.tile([B_LOC, CH, L], f32, tag="emit")
                if first_chunk:
                    nc.sync.dma_start(out=emit_sb[:], in_=emit_d[:, 0:CH, :])
                else:
                    nc.sync.dma_start(out=emit_sb[:],
                                      in_=emit_d[:, bass.ds(iv, CH), :])
                trell_sb = tpool.tile([L, CH, B_LOC], u8, tag="trell")
                for tl in range(CH):
                    do_step(tl, emit_sb, trell_sb)
                if first_chunk:
                    nc.sync.dma_start(out=trell_d[:, 0:CH, :], in_=trell_sb[:])
                else:
                    nc.sync.dma_start(out=trell_d[:, bass.ds(iv, CH), :],
                                      in_=trell_sb[:])

            do_chunk(0, True)
            if n_chunks > 1:
                if dynamic:
                    with tc.For_i(CH, T, CH, staggered_reset=True) as iv:
                        do_chunk(iv, False)
                else:
                    for c in range(1, n_chunks):
                        do_chunk(c * CH, False)

            # after the last step vT already equals v_final
            nc.sync.dma_start(out=vfin_d[:], in_=vmaxT[:])

    nc.compile()
    return nc


def _strip_pe_own_waits(nc):
    """Remove Matmult waits on PE's own semaphore that are provably satisfied
    by program order.  Valid only for a fully static (loop-free) module: walk
    blocks in order, count each semaphore's increments, and drop a same-engine
    wait whose target value has already been reached earlier in the program.
    Walrus codegen only supports a single sync-wait per Matmult; Tile emits a
    redundant PE-own wait on PSUM slot reuse (in-order engine => always met).
    """
    import concourse.mybir as mybir

    counts = {}
    n_stripped = 0
    n_multi = 0
    for blk in nc.m.functions[0].blocks:
        for inst in blk.instructions:
            si = inst.sync_info
            if si is not None and si.on_wait:
                if type(inst).__name__ == "InstMatmult":
                    eng = str(inst.engine).split(".")[-1]
                    keep = []
                    for w in si.on_wait:
                        own = w.ant_name.split("_")[0] == eng
                        if (own and str(w.wait_mode).endswith("sem-ge-imm")
                                and counts.get(w.ant_name, 0) >= w.wait_value):
                            n_stripped += 1
                        else:
                            keep.append(w)
                    if len(keep) != len(si.on_wait):
                        inst.sync_info = mybir.SyncInfo(
                            on_wait=keep, on_update=list(si.on_update))
                    if len(keep) > 1:
                        n_multi += 1
            si = inst.sync_info
            if si is not None:
                for u in si.on_update:
                    if str(u.update_mode).endswith("sem-inc"):
                        counts[u.ant_name] = counts.get(u.ant_name, 0) + u.update_value
    return n_stripped, n_multi


def _get_nc(T=SEQ, CH=64, dynamic=False):
    key = (T, CH, dynamic)
    if key not in _CACHE:
        _CACHE[key] = _build(T, CH, dynamic)
    return _CACHE[key]


def _sel_matrix():
    sel = np.zeros((B_LOC, B_LOC * NUM_LABELS), dtype=np.float32)
    for b in range(B_LOC):
        sel[b, b * NUM_LABELS:(b + 1) * NUM_LABELS] = 1.0
    return sel


class _Runner:
    """Builds the Bass module and a cached sharded jit over the 8 cores.

    Mirrors concourse.bass2jax.run_bass_via_pjrt's multi-core path, but keeps
    the jitted callable so repeat invocations skip retrace/recompile.
    """

    def __init__(self, T):
        import jax
        import numpy as _np
        from jax.sharding import Mesh, PartitionSpec
        from jax.experimental.shard_map import shard_map
        import concourse.mybir as mybir
        from concourse import bass2jax

        bass2jax.install_neuronx_cc_hook()
        nc = _get_nc(T=T)
        self.nc = nc
        partition_name = (nc.partition_id_tensor.name
                          if nc.partition_id_tensor else None)
        in_names, out_names, out_avals = [], [], []
        for alloc in nc.m.functions[0].allocations:
            if not isinstance(alloc, mybir.MemoryLocationSet):
                continue
            name = alloc.memorylocations[0].name
            if alloc.kind == "ExternalInput":
                if name != partition_name:
                    in_names.append(name)
            elif alloc.kind == "ExternalOutput":
                out_names.append(name)
                out_avals.append(jax.core.ShapedArray(
                    tuple(alloc.tensor_shape), mybir.dt.np(alloc.dtype)))
        self.in_names, self.out_names, self.out_avals = in_names, out_names, out_avals
        n_params = len(in_names)
        bind_names = list(in_names + out_names)
        if partition_name is not None:
            bind_names.append(partition_name)
        bind_names = tuple(bind_names)

        def _body(*args):
            operands = list(args)
            if partition_name is not None:
                operands.append(bass2jax.partition_id_tensor())
            outs = bass2jax._bass_exec_p.bind(
                *operands,
                out_avals=tuple(out_avals),
                in_names=bind_names,
                out_names=tuple(out_names),
                lowering_input_output_aliases=(),
                sim_require_finite=True,
                sim_require_nnan=True,
                nc=nc,
            )
            return tuple(outs)

        devices = jax.devices()[:N_CORES]
        self.mesh = Mesh(_np.asarray(devices), ("core",))
        n_outs = len(out_names)
        in_specs = (PartitionSpec("core"),) * (n_params + n_outs)
        out_specs = (PartitionSpec("core"),) * n_outs
        self.jit = jax.jit(
            shard_map(_body, mesh=self.mesh, in_specs=in_specs,
                      out_specs=out_specs, check_rep=False),
            donate_argnums=tuple(range(n_params, n_params + n_outs)),
            keep_unused=True,
        )

    def run(self, per_core_inputs):
        """per_core_inputs: list (len 8) of dicts name->np array."""
        import numpy as _np
        concat_in = [
            _np.concatenate([per_core_inputs[c][n] for c in range(N_CORES)], axis=0)
            for n in self.in_names
        ]
        zeros = [
            _np.zeros((N_CORES * a.shape[0], *a.shape[1:]), a.dtype)
            for a in self.out_avals
        ]
        out_arrs = self.jit(*concat_in, *zeros)
        return {
            n: _np.asarray(out_arrs[i]).reshape(
                N_CORES, *self.out_avals[i].shape)
            for i, n in enumerate(self.out_names)
        }


_RUNNERS = {}


def _get_runner(T):
    if T not in _RUNNERS:
        _RUNNERS[T] = _Runner(T)
    return _RUNNERS[T]


def kernel(inputs: np.ndarray, trans: np.ndarray) -> tuple:
    inputs = np.ascontiguousarray(inputs, dtype=np.float32)
    trans = np.ascontiguousarray(trans, dtype=np.float32)
    B, T, L = inputs.shape
    transT = np.ascontiguousarray(trans.T)
    ident = np.eye(L, dtype=np.float32)
    sel = _sel_matrix()

    runner = _get_runner(T)
    transT4 = np.ascontiguousarray(np.tile(transT, (1, 4)))
    per_core = [
        {"emit": inputs[c * B_LOC:(c + 1) * B_LOC], "transT": transT,
         "sel": sel, "ident": ident, "transT4": transT4}
        for c in range(N_CORES)
    ]
    outs = runner.run(per_core)

    # host epilogue: final max/argmax + backtrack (exact first-index ties)
    vfin = outs["vfin"].reshape(B, L)
    scores = vfin.max(axis=1)
    ptr = vfin.argmax(axis=1).astype(np.int64)
    # trell: [8, 128, T, 32] -> [128, T, 256] (core-major batch order)
    trell = np.moveaxis(outs["trell"], 0, 2).reshape(NUM_LABELS, T, B)
    paths = np.empty((B, T), dtype=np.int32)
    ar = np.arange(B)
    for t in range(T - 1, -1, -1):
        ptr = trell[ptr, t, ar].astype(np.int64)
        paths[:, t] = ptr
    return inputs, scores, paths
